# revision 29
# baseline (speedup 1.0000x reference)
"""Deformable conv block (nn_DeformableConvBlock) Trainium2 Bass kernel.

Math: offset = conv3x3(x, w_off) + b_off; bilinear-sample x at
p0 + k + offset per tap; out = einsum over (c, tap) with w_def + b_def.

Device algorithm (per NeuronCore, data-parallel over batch x row-halves):
gather-free bilinear via the dense hat expansion
  samp = sum_{s,t} hat(dy - s) * hat(dx - t) * x[p + k + (s,t)],
  hat(t) = relu(1 - |t|), lags s,t in {-2..2} (covers |offset| < 2).
Both hat factors are per-DEST-pixel maps, so per horizontal lag t a
vertical pass V_t = sum_s vm_s * X[row+s, col+u(t)] accumulates
dest-aligned products (DVE multiplies, Pool adds; the column shift is
folded into the X operand read), then one hm_t multiply per t feeds the
PE contraction (PSUM accumulates taps and t exactly in f32).

The whole chunk pipeline sits inside a hardware For_i loop (8 chunks of 8
output rows), software-pipelined one chunk ahead (staging DMA + offset
conv + hat maps for chunk i+1 overlap the multiplies of chunk i).  Hat
maps are computed compactly on [18, cols] tiles (ACT: Abs then
Relu(1-x)) and replicated across channel partitions with broadcast DMAs.

Layout: zero-padded bf16 row slabs [64c, 14 rows, 136 cols] staged per
chunk; taps run partition-stacked in pairs (128-wide ops) using
row/col-shifted upper-half slab copies.  f32 I/O, bf16 compute, f32 PSUM.
"""

import sys

sys.path.insert(0, "/opt/trn_rl_repo")

import numpy as np
import ml_dtypes

import concourse.bass as bass
import concourse.mybir as mybir
import concourse.tile as tile
from concourse import bass_utils
from concourse.bass import ds

BF = ml_dtypes.bfloat16

B, C, H, W = 4, 64, 128, 128
CO, KK = 64, 9
LAGS = (-2, -1, 0, 1, 2)
NLAG = 5
RH = 64            # output rows per core
CH = 8             # output rows per chunk
NCH = RH // CH
WP = 136           # padded width (4 zero cols each side)
XROWS = 80         # DRAM slab rows (72 real + tail pad for prefetch)
SROWS = 14         # staged slab rows per chunk
NFREE = CH * WP    # 1088 full-width elements per chunk
NINT = CH * W      # 1024 interior elements per chunk
NF = NFREE - 8     # interior + shift margin
NDQ = CH * W       # dense interior elements (8 rows x 128 cols)

# (taps, X source, ky, u_base, Pn, which lhsT)
#   XB: upper half = +1 row;  XC: upper half = +1 col
GROUPS = (
    ((0, 3), "XB", 0, -1, 128),
    ((1, 4), "XB", 0, 0, 128),
    ((2, 5), "XB", 0, 1, 128),
    ((6, 7), "XC", 2, -1, 128),
    ((8,), "XBl", 2, 1, 64),
)

bf16 = mybir.dt.bfloat16
f32 = mybir.dt.float32
MUL = mybir.AluOpType.mult
ADD = mybir.AluOpType.add
MAX = mybir.AluOpType.max
AF = mybir.ActivationFunctionType

SEGS = ((0, 512), (512, 1024), (1024, NFREE))


def _sl(base, ln):
    """Static slice for int base, DynSlice for ScalarValue base."""
    if isinstance(base, int):
        return slice(base, base + ln)
    return ds(base, ln)


def build_program(rep=1, pool_adds=0, act_relu=False,
                  do_mm=True, do_mul=True, do_bc=True, do_prep=True):
    nc = bass.Bass("TRN2", target_bir_lowering=False, debug=False)

    xs = nc.dram_tensor("xs", [64, XROWS * WP], bf16, kind="ExternalInput")
    woffA = nc.dram_tensor("woffA", [128, 3 * 18], bf16, kind="ExternalInput")
    woffB = nc.dram_tensor("woffB", [64, 3 * 18], bf16, kind="ExternalInput")
    hbias = nc.dram_tensor("hbias", [18, NLAG], f32, kind="ExternalInput")
    wdefP = nc.dram_tensor("wdefP", [128, 4 * 64], bf16, kind="ExternalInput")
    wdef8 = nc.dram_tensor("wdef8", [65, 2 * 64], bf16, kind="ExternalInput")
    yout = nc.dram_tensor("y", [64, RH * W], f32, kind="ExternalOutput")

    with tile.TileContext(nc) as tc:
        with tc.tile_pool(name="cst", bufs=1) as cst, \
             tc.tile_pool(name="sx", bufs=1) as sx, \
             tc.tile_pool(name="pr", bufs=1) as pr, \
             tc.tile_pool(name="meg", bufs=1) as meg, \
             tc.tile_pool(name="vp", bufs=4) as vp, \
             tc.tile_pool(name="tp", bufs=5) as tp, \
             tc.tile_pool(name="qp", bufs=2) as qp, \
             tc.tile_pool(name="oep", bufs=2) as oep, \
             tc.tile_pool(name="pso", bufs=1, space="PSUM") as pso, \
             tc.tile_pool(name="psc", bufs=1, space="PSUM") as psc:

            twoffA = cst.tile([128, 3 * 18], bf16, tag="twoffA")
            twoffB = cst.tile([64, 3 * 18], bf16, tag="twoffB")
            thb = cst.tile([18, NLAG], f32, tag="thb")
            twdefP = cst.tile([128, 4 * 64], bf16, tag="twdefP")
            twdef8 = cst.tile([65, 2 * 64], bf16, tag="twdef8")
            q8 = cst.tile([65, NDQ], bf16, tag="q8")

            nc.sync.dma_start(twoffA[:], woffA.ap())
            nc.sync.dma_start(twoffB[:], woffB.ap())
            nc.sync.dma_start(thb[:], hbias.ap())
            nc.sync.dma_start(twdefP[:], wdefP.ap())
            nc.sync.dma_start(twdef8[:], wdef8.ap())
            nc.vector.memset(q8[64:65, :], 1.0)

            XB = [sx.tile([128, SROWS * WP + 8], bf16, tag=f"XB{p}", name=f"XB{p}") for p in (0, 1)]
            XC = [sx.tile([128, SROWS * WP], bf16, tag=f"XC{p}", name=f"XC{p}") for p in (0, 1)]
            mabs = [pr.tile([18, NFREE], bf16, tag=f"mabs{p}", name=f"mabs{p}") for p in (0, 1)]
            mc = [pr.tile([18, NLAG * NDQ], bf16, tag=f"mc{p}", name=f"mc{p}") for p in (0, 1)]
            mcs = pr.tile([120, NLAG * NDQ], bf16, tag="mcs")
            ps2 = [psc.tile([18, NFREE], f32, tag=f"ps2{p}", name=f"ps2{p}") for p in (0, 1)]
            vmeg = [meg.tile([128, NLAG * NDQ], bf16, tag=f"vm{g}", name=f"vm{g}")
                    for g in range(4)]
            hmeg = [meg.tile([128, NLAG * NDQ], bf16, tag=f"hm{g}", name=f"hm{g}")
                    for g in range(4)]
            vmeg8 = meg.tile([64, NLAG * NDQ], bf16, tag="vm8")
            hmeg8 = meg.tile([64, NLAG * NDQ], bf16, tag="hm8")

            def prep(ci, p):
                """Stage chunk ci's slab rows + offset conv + compact hat maps."""
                base = ci * (CH * WP) if isinstance(ci, int) else ci * (CH * WP)
                L = SROWS * WP
                xb, xc = XB[p], XC[p]
                xsa = xs.ap()
                nc.sync.dma_start(xb[0:64, 0:L + 8], xsa[:, _sl(base, L + 8)])
                nc.sync.dma_start(xb[64:128, 0:L], xsa[:, _sl(base + WP, L)])
                # XC halves are static copies of XB-lower (same rows; upper
                # shifted one column) -- no dynamic-AP registers needed
                nc.scalar.dma_start(xc[0:64, 0:L], xb[0:64, 0:L])
                nc.scalar.dma_start(xc[64:128, 0:L], xb[0:64, 1:1 + L])

                p2 = ps2[p]
                for a, b in (SEGS if do_prep else ()):
                    for ctx in range(3):
                        nc.tensor.matmul(
                            p2[:, a:b],
                            twoffA[:, ctx * 18:(ctx + 1) * 18],
                            xb[0:128, 2 * WP + ctx - 1 + a:
                               2 * WP + ctx - 1 + a + (b - a)],
                            start=(ctx == 0), stop=False)
                        nc.tensor.matmul(
                            p2[:, a:b],
                            twoffB[:, ctx * 18:(ctx + 1) * 18],
                            xb[0:64, 4 * WP + ctx - 1 + a:
                               4 * WP + ctx - 1 + a + (b - a)],
                            start=False, stop=(ctx == 2))

                for si in (range(NLAG) if do_prep else ()):
                    nc.scalar.activation(mabs[p][:], p2[:], AF.Abs,
                                         bias=thb[:, si:si + 1], scale=1.0)
                    m3 = mabs[p][:].rearrange("p (r w) -> p r w", w=WP)
                    dst = mc[p][:, si * NDQ:(si + 1) * NDQ].rearrange(
                        "p (r w) -> p r w", w=W)
                    if act_relu:
                        nc.scalar.activation(dst, m3[:, 0:CH, 4:4 + W],
                                             AF.Relu, bias=1.0, scale=-1.0)
                    else:
                        nc.gpsimd.tensor_scalar(dst, m3[:, 0:CH, 4:4 + W],
                                                -1.0, 1.0, MUL, ADD)
                        nc.gpsimd.tensor_scalar(dst, dst, 0.0, None, MAX)

            _ring = [0]

            def bcast(p):
                """Replicate compact hat maps across channel partitions.

                Tree broadcast: a flat [0,64]-stride replica DMA is bound by
                the ONE source SBUF port holding the map row (~27 GB/s).
                Stage 0 spreads the 18 rows across partitions (ports), then
                per map: 1->16 copies at partition stride 4, then 3 gap-fill
                DMAs whose reads are spread over 16 partitions each."""
                N = NLAG * NDQ
                mca = mc[p][:]
                msa = mcs[:]
                mstep = msa.ap[0][0]
                if do_bc:
                    eng = nc.sync if _ring[0] % 2 == 0 else nc.scalar
                    _ring[0] += 1
                    eng.dma_start(
                        bass.AP(msa.tensor, msa.offset, [[7 * mstep, 18], [1, N]]),
                        bass.AP(mca.tensor, mca.offset,
                                [[mca.ap[0][0], 18], [1, N]]))

                def one(mega, p0, row):
                    ma = mega[:]
                    pstep = ma.ap[0][0]
                    eng = nc.sync if _ring[0] % 2 == 0 else nc.scalar
                    _ring[0] += 1
                    if not do_bc:
                        return
                    # stage 1: mcs row -> 16 copies at partition stride 4
                    src1 = bass.AP(msa.tensor, msa.offset + 7 * row * mstep,
                                   [[mstep, 1], [0, 16], [1, N]])
                    dst1 = bass.AP(ma.tensor, ma.offset + p0 * pstep,
                                   [[4 * pstep, 16], [0, 1], [1, N]])
                    eng.dma_start(dst1, src1)
                    # stage 2: fill j=1..3 within each stride-4 block
                    for j in (1, 2, 3):
                        eng = nc.sync if _ring[0] % 2 == 0 else nc.scalar
                        _ring[0] += 1
                        src2 = bass.AP(ma.tensor, ma.offset + p0 * pstep,
                                       [[4 * pstep, 16], [0, 1], [1, N]])
                        dst2 = bass.AP(ma.tensor,
                                       ma.offset + (p0 + j) * pstep,
                                       [[4 * pstep, 16], [0, 1], [1, N]])
                        eng.dma_start(dst2, src2)

                for g, (taps, _, _, _, _) in enumerate(GROUPS[:4]):
                    kA, kB = taps
                    one(vmeg[g], 0, 2 * kA)
                    one(vmeg[g], 64, 2 * kB)
                    one(hmeg[g], 0, 2 * kA + 1)
                    one(hmeg[g], 64, 2 * kB + 1)
                one(vmeg8, 0, 16)
                one(hmeg8, 0, 17)

            def mult(ci, p):
                """Per-lag-t vertical hat pass (weights dest-aligned, column
                shift folded into the X read), one hm multiply, PE sums t."""
                xsrc = {"XB": XB[p][0:128, 0:SROWS * WP],
                        "XC": XC[p][0:128, 0:SROWS * WP],
                        "XBl": XB[p][0:64, 0:SROWS * WP]}
                vms = [vmeg[g] if g < 4 else vmeg8 for g in range(5)]
                hms = [hmeg[g] if g < 4 else hmeg8 for g in range(5)]

                pot = pso.tile([64, NINT], f32, tag="pot")
                nmm = 2 * NLAG * len(GROUPS)
                mmi = 0
                pool_si = set(range(5 - pool_adds, 5)) if pool_adds else set()
                for g, (taps, xk, ky, ub, Pn) in enumerate(GROUPS):
                    for ti, t in enumerate(LAGS):
                        X = xsrc[xk]
                        X3 = X.rearrange("p (r w) -> p r w", w=WP)
                        vm, hm = vms[g], hms[g]
                        u = ub + t
                        Vt = vp.tile([128, NDQ], bf16, tag="V", name="V")
                        V3 = Vt[:].rearrange("p (r w) -> p r w", w=W)
                        tmps = {}
                        if not do_mul:
                            continue
                        for si in sorted(pool_si):
                            r0 = 2 + ky + LAGS[si]
                            tt = tp.tile([128, NDQ], bf16, tag="tmp")
                            t3 = tt[:].rearrange("p (r w) -> p r w", w=W)
                            nc.gpsimd.tensor_tensor(
                                t3[0:Pn],
                                vm[0:Pn, si * NDQ:(si + 1) * NDQ].rearrange(
                                    "p (r w) -> p r w", w=W),
                                X3[0:Pn, r0:r0 + CH, 4 + u:4 + u + W], MUL)
                            tmps[si] = tt
                        first = True
                        for si in range(NLAG):
                            r0 = 2 + ky + LAGS[si]
                            vm3 = vm[0:Pn, si * NDQ:(si + 1) * NDQ].rearrange(
                                "p (r w) -> p r w", w=W)
                            if first and si not in pool_si:
                                nc.vector.tensor_tensor(
                                    V3[0:Pn], vm3,
                                    X3[0:Pn, r0:r0 + CH, 4 + u:4 + u + W], MUL)
                                first = False
                                continue
                            if si in pool_si:
                                tt = tmps[si]
                            else:
                                tt = tp.tile([128, NDQ], bf16, tag="tmp")
                                nc.vector.tensor_tensor(
                                    tt[:].rearrange("p (r w) -> p r w", w=W)[0:Pn],
                                    vm3,
                                    X3[0:Pn, r0:r0 + CH, 4 + u:4 + u + W], MUL)
                            nc.vector.tensor_tensor(Vt[0:Pn, :], Vt[0:Pn, :],
                                                    tt[0:Pn, :], ADD)
                        if g == 4:
                            q = q8
                            lhs = twdef8[:, 0:64] if ti == 0 else twdef8[:, 64:128]
                            Pq = 65
                        else:
                            q = qp.tile([128, NDQ], bf16, tag="q")
                            lhs = twdefP[:, g * 64:(g + 1) * 64]
                            Pq = Pn
                        nc.vector.tensor_tensor(
                            q[0:Pn, :],
                            hm[0:Pn, ti * NDQ:(ti + 1) * NDQ],
                            Vt[0:Pn, :], MUL)
                        qr = q[:].rearrange("p (r w) -> p r w", w=W)
                        for colh in range(2 if do_mm else 0):
                            nc.tensor.matmul(
                                pot[:, colh * 512:(colh + 1) * 512],
                                lhs,
                                qr[0:Pq, colh * (CH // 2):(colh + 1) * (CH // 2), :],
                                start=(mmi < 2), stop=(mmi >= nmm - 2))
                            mmi += 1
                        if not do_mm:
                            mmi += 2

                if do_mm and do_mul:
                    oe = oep.tile([64, NINT], f32, tag="oe")
                    nc.scalar.activation(oe[:], pot[:], AF.Copy,
                                         bias=0.0, scale=1.0)
                    yo = ci * NINT if isinstance(ci, int) else ci * NINT
                    nc.scalar.dma_start(yout.ap()[:, _sl(yo, NINT)], oe[:])

            with tc.For_i(0, rep, 1):
                prep(0, 0)
                with tc.For_i(0, NCH, 2,
                              hint_engines=(mybir.EngineType.DVE,)) as i:
                    for k in range(2):
                        bcast(k % 2)
                        prep(i + k + 1, (k + 1) % 2)
                        mult(i + k, k % 2)

    return nc


def _split_multiwait(nc, maxw=1):
    """This container's walrus rejects >1 sync-wait per instruction; hoist
    extra waits onto preceding NoOps."""
    n_new = 0
    for f in nc.m.functions:
        for bb in f.blocks:
            out = []
            changed = False
            for ins in bb.instructions:
                si = getattr(ins, "sync_info", None)
                if si is not None and si.on_wait and len(si.on_wait) > maxw:
                    waits = list(si.on_wait)
                    hoist, keep = waits[:-maxw], waits[-maxw:]
                    for i in range(0, len(hoist), maxw):
                        nop = mybir.InstNoOp(
                            name=f"I-waitsplit-{n_new}",
                            sync_info=mybir.SyncInfo(on_wait=hoist[i:i + maxw],
                                                     on_update=[]),
                            bass_nofuse=True,
                            engine=ins.engine)
                        n_new += 1
                        out.append(nop)
                    ins.sync_info = mybir.SyncInfo(on_wait=keep,
                                                  on_update=list(si.on_update))
                    changed = True
                out.append(ins)
            if changed:
                bb.instructions = out
    return n_new


_PROGRAM_CACHE = {}


def _get_program(rep=1):
    if rep not in _PROGRAM_CACHE:
        nc = build_program(rep=rep)
        _split_multiwait(nc)
        _PROGRAM_CACHE[rep] = nc
    return _PROGRAM_CACHE[rep]


def _host_pack(x, w_off, b_off, w_def, b_def):
    slabs = np.zeros((8, 64, XROWS, WP), BF)
    for i in range(8):
        b, r0 = i // 2, (i % 2) * RH
        lo = r0 - 3
        s_lo, s_hi = max(lo, 0), min(lo + 72, H)
        slabs[i, :, s_lo - lo:s_hi - lo, 4:4 + W] = x[b, :, s_lo:s_hi, :].astype(BF)

    wof = w_off.reshape(18, 64, 3, 3)
    woffA = np.zeros((128, 3, 18), BF)
    woffB = np.zeros((64, 3, 18), BF)
    for ctx in range(3):
        woffA[:64, ctx, :] = wof[:, :, 0, ctx].T.astype(BF)
        woffA[64:, ctx, :] = wof[:, :, 1, ctx].T.astype(BF)
        woffB[:, ctx, :] = wof[:, :, 2, ctx].T.astype(BF)

    hb = np.zeros((18, NLAG), np.float32)
    for si, s in enumerate(LAGS):
        hb[:, si] = b_off - s

    wd = w_def.reshape(CO, C, KK)
    wdefP = np.zeros((128, 4, 64), BF)
    for g, (taps, _, _, _, _) in enumerate(GROUPS[:4]):
        kA, kB = taps
        wdefP[:64, g, :] = wd[:, :, kA].T.astype(BF)
        wdefP[64:, g, :] = wd[:, :, kB].T.astype(BF)
    wdef8 = np.zeros((65, 2, 64), BF)
    wdef8[:64, 0, :] = wd[:, :, 8].T.astype(BF)
    wdef8[64, 0, :] = b_def.astype(BF)
    wdef8[:64, 1, :] = wd[:, :, 8].T.astype(BF)

    return [{
        "xs": np.ascontiguousarray(slabs[i].reshape(64, XROWS * WP)),
        "woffA": np.ascontiguousarray(woffA.reshape(128, 54)),
        "woffB": np.ascontiguousarray(woffB.reshape(64, 54)),
        "hbias": hb,
        "wdefP": np.ascontiguousarray(wdefP.reshape(128, 256)),
        "wdef8": np.ascontiguousarray(wdef8.reshape(65, 128)),
    } for i in range(8)]


def kernel(x, w_off, b_off, w_def, b_def):
    x = np.asarray(x, np.float32)
    w_off = np.asarray(w_off, np.float32)
    b_off = np.asarray(b_off, np.float32)
    w_def = np.asarray(w_def, np.float32)
    b_def = np.asarray(b_def, np.float32)

    nc = _get_program(rep=1)
    in_maps = _host_pack(x, w_off, b_off, w_def, b_def)
    res = bass_utils.run_bass_kernel_spmd(nc, in_maps, core_ids=list(range(8)))

    y = np.zeros((B, CO, H, W), np.float32)
    for i in range(8):
        b, r0 = i // 2, (i % 2) * RH
        y[b, :, r0:r0 + RH, :] = res.results[i]["y"].reshape(CO, RH, W)
    return y


if __name__ == "__main__":
    import reference as R
    inp = {k: np.asarray(v, np.float32) for k, v in R.setup_inputs().items()}
    y = kernel(**inp)
    print("out", y.shape, y.dtype, float(np.abs(y).max()))


# revision 30
# speedup vs baseline: 1.4343x; 1.4343x over previous
"""Deformable conv block (nn_DeformableConvBlock) Trainium2 Bass kernel.

Math: offset = conv3x3(x, w_off) + b_off; bilinear-sample x at
p0 + k + offset per tap; out = einsum over (c, tap) with w_def + b_def.

Device algorithm (per NeuronCore, data-parallel over batch x row-halves):
gather-free bilinear via the dense hat expansion
  samp = sum_{s,t} hat(dy - s) * hat(dx - t) * x[p + k + (s,t)],
  hat(t) = relu(1 - |t|), lags s,t in {-2..2} (covers |offset| < 2).
Both hat factors are per-DEST-pixel maps, so per horizontal lag t a
vertical pass V_t = sum_s vm_s * X[row+s, col+u(t)] accumulates
dest-aligned products (DVE multiplies, Pool adds; the column shift is
folded into the X operand read), then one hm_t multiply per t feeds the
PE contraction (PSUM accumulates taps and t exactly in f32).

The whole chunk pipeline sits inside a hardware For_i loop (8 chunks of 8
output rows), software-pipelined one chunk ahead (staging DMA + offset
conv + hat maps for chunk i+1 overlap the multiplies of chunk i).  Hat
maps are computed compactly on [18, cols] tiles (ACT: Abs then
Relu(1-x)) and replicated across channel partitions with broadcast DMAs.

Layout: zero-padded bf16 row slabs [64c, 14 rows, 136 cols] staged per
chunk; taps run partition-stacked in pairs (128-wide ops) using
row/col-shifted upper-half slab copies.  f32 I/O, bf16 compute, f32 PSUM.
"""

import sys

sys.path.insert(0, "/opt/trn_rl_repo")

import numpy as np
import ml_dtypes

import concourse.bass as bass
import concourse.mybir as mybir
import concourse.tile as tile
from concourse import bass_utils
from concourse.bass import ds

BF = ml_dtypes.bfloat16

B, C, H, W = 4, 64, 128, 128
CO, KK = 64, 9
LAGS = (-2, -1, 0, 1, 2)
NLAG = 5
RH = 64            # output rows per core
CH = 8             # output rows per chunk
NCH = RH // CH
WP = 136           # padded width (4 zero cols each side)
XROWS = 80         # DRAM slab rows (72 real + tail pad for prefetch)
SROWS = 14         # staged slab rows per chunk
NFREE = CH * WP    # 1088 full-width elements per chunk
NINT = CH * W      # 1024 interior elements per chunk
NF = NFREE - 8     # interior + shift margin
NDQ = CH * W       # dense interior elements (8 rows x 128 cols)

# (taps, X source, ky, u_base, Pn, which lhsT)
#   XB: upper half = +1 row;  XC: upper half = +1 col
GROUPS = (
    ((0, 3), "XB", 0, -1, 128),
    ((1, 4), "XB", 0, 0, 128),
    ((2, 5), "XB", 0, 1, 128),
    ((6, 7), "XC", 2, -1, 128),
    ((8,), "XBl", 2, 1, 64),
)

bf16 = mybir.dt.bfloat16
f32 = mybir.dt.float32
MUL = mybir.AluOpType.mult
ADD = mybir.AluOpType.add
MAX = mybir.AluOpType.max
AF = mybir.ActivationFunctionType

SEGS = ((0, 512), (512, 1024), (1024, NFREE))


def _sl(base, ln):
    """Static slice for int base, DynSlice for ScalarValue base."""
    if isinstance(base, int):
        return slice(base, base + ln)
    return ds(base, ln)


def build_program(rep=1, pool_adds=0, act_relu=True,
                  do_mm=True, do_mul=True, do_bc=True, do_prep=True):
    nc = bass.Bass("TRN2", target_bir_lowering=False, debug=False)

    xs = nc.dram_tensor("xs", [64, XROWS * WP], bf16, kind="ExternalInput")
    woffA = nc.dram_tensor("woffA", [128, 3 * 18], bf16, kind="ExternalInput")
    woffB = nc.dram_tensor("woffB", [64, 3 * 18], bf16, kind="ExternalInput")
    hbias = nc.dram_tensor("hbias", [18, NLAG], f32, kind="ExternalInput")
    wdefP = nc.dram_tensor("wdefP", [128, 4 * 64], bf16, kind="ExternalInput")
    wdef8 = nc.dram_tensor("wdef8", [65, 2 * 64], bf16, kind="ExternalInput")
    yout = nc.dram_tensor("y", [64, RH * W], f32, kind="ExternalOutput")

    with tile.TileContext(nc) as tc:
        with tc.tile_pool(name="cst", bufs=1) as cst, \
             tc.tile_pool(name="sx", bufs=1) as sx, \
             tc.tile_pool(name="pr", bufs=1) as pr, \
             tc.tile_pool(name="meg", bufs=1) as meg, \
             tc.tile_pool(name="vp", bufs=4) as vp, \
             tc.tile_pool(name="tp", bufs=5) as tp, \
             tc.tile_pool(name="qp", bufs=2) as qp, \
             tc.tile_pool(name="oep", bufs=2) as oep, \
             tc.tile_pool(name="pso", bufs=1, space="PSUM") as pso, \
             tc.tile_pool(name="psc", bufs=1, space="PSUM") as psc:

            twoffA = cst.tile([128, 3 * 18], bf16, tag="twoffA")
            twoffB = cst.tile([64, 3 * 18], bf16, tag="twoffB")
            thb = cst.tile([18, NLAG], f32, tag="thb")
            twdefP = cst.tile([128, 4 * 64], bf16, tag="twdefP")
            twdef8 = cst.tile([65, 2 * 64], bf16, tag="twdef8")
            q8 = cst.tile([65, NDQ], bf16, tag="q8")

            nc.sync.dma_start(twoffA[:], woffA.ap())
            nc.sync.dma_start(twoffB[:], woffB.ap())
            nc.sync.dma_start(thb[:], hbias.ap())
            nc.sync.dma_start(twdefP[:], wdefP.ap())
            nc.sync.dma_start(twdef8[:], wdef8.ap())
            nc.vector.memset(q8[64:65, :], 1.0)

            XB = [sx.tile([128, SROWS * WP + 8], bf16, tag=f"XB{p}", name=f"XB{p}") for p in (0, 1)]
            XC = [sx.tile([128, SROWS * WP], bf16, tag=f"XC{p}", name=f"XC{p}") for p in (0, 1)]
            mabs = [pr.tile([18, NFREE], bf16, tag=f"mabs{p}", name=f"mabs{p}") for p in (0, 1)]
            mc = [pr.tile([18, NLAG * NDQ], bf16, tag=f"mc{p}", name=f"mc{p}") for p in (0, 1)]
            mcs = pr.tile([120, NLAG * NDQ], bf16, tag="mcs")
            ps2 = [psc.tile([18, NFREE], f32, tag=f"ps2{p}", name=f"ps2{p}") for p in (0, 1)]
            vmeg = [meg.tile([128, NLAG * NDQ], bf16, tag=f"vm{g}", name=f"vm{g}")
                    for g in range(4)]
            hmeg = [meg.tile([128, NLAG * NDQ], bf16, tag=f"hm{g}", name=f"hm{g}")
                    for g in range(4)]
            vmeg8 = meg.tile([64, NLAG * NDQ], bf16, tag="vm8")
            hmeg8 = meg.tile([64, NLAG * NDQ], bf16, tag="hm8")

            def prep(ci, p):
                """Stage chunk ci's slab rows + offset conv + compact hat maps."""
                base = ci * (CH * WP) if isinstance(ci, int) else ci * (CH * WP)
                L = SROWS * WP
                xb, xc = XB[p], XC[p]
                xsa = xs.ap()
                nc.sync.dma_start(xb[0:64, 0:L + 8], xsa[:, _sl(base, L + 8)])
                nc.sync.dma_start(xb[64:128, 0:L], xsa[:, _sl(base + WP, L)])
                # XC halves are static copies of XB-lower (same rows; upper
                # shifted one column) -- no dynamic-AP registers needed
                nc.scalar.dma_start(xc[0:64, 0:L], xb[0:64, 0:L])
                nc.scalar.dma_start(xc[64:128, 0:L], xb[0:64, 1:1 + L])

                p2 = ps2[p]
                for a, b in (SEGS if do_prep else ()):
                    for ctx in range(3):
                        nc.tensor.matmul(
                            p2[:, a:b],
                            twoffA[:, ctx * 18:(ctx + 1) * 18],
                            xb[0:128, 2 * WP + ctx - 1 + a:
                               2 * WP + ctx - 1 + a + (b - a)],
                            start=(ctx == 0), stop=False)
                        nc.tensor.matmul(
                            p2[:, a:b],
                            twoffB[:, ctx * 18:(ctx + 1) * 18],
                            xb[0:64, 4 * WP + ctx - 1 + a:
                               4 * WP + ctx - 1 + a + (b - a)],
                            start=False, stop=(ctx == 2))

                for si in (range(NLAG) if do_prep else ()):
                    nc.scalar.activation(mabs[p][:], p2[:], AF.Abs,
                                         bias=thb[:, si:si + 1], scale=1.0)
                    m3 = mabs[p][:].rearrange("p (r w) -> p r w", w=WP)
                    dst = mc[p][:, si * NDQ:(si + 1) * NDQ].rearrange(
                        "p (r w) -> p r w", w=W)
                    if act_relu:
                        nc.scalar.activation(dst, m3[:, 0:CH, 4:4 + W],
                                             AF.Relu, bias=1.0, scale=-1.0)
                    else:
                        nc.vector.tensor_scalar(dst, m3[:, 0:CH, 4:4 + W],
                                                -1.0, 1.0, MUL, ADD)
                        nc.vector.tensor_scalar(dst, dst, 0.0, None, MAX)

            _ring = [0]

            def bcast(p):
                """Replicate compact hat maps across channel partitions.

                Tree broadcast: a flat [0,64]-stride replica DMA is bound by
                the ONE source SBUF port holding the map row (~27 GB/s).
                Stage 0 spreads the 18 rows across partitions (ports), then
                per map: 1->16 copies at partition stride 4, then 3 gap-fill
                DMAs whose reads are spread over 16 partitions each."""
                N = NLAG * NDQ
                mca = mc[p][:]
                msa = mcs[:]
                mstep = msa.ap[0][0]
                if do_bc:
                    eng = nc.sync if _ring[0] % 2 == 0 else nc.scalar
                    _ring[0] += 1
                    eng.dma_start(
                        bass.AP(msa.tensor, msa.offset, [[7 * mstep, 18], [1, N]]),
                        bass.AP(mca.tensor, mca.offset,
                                [[mca.ap[0][0], 18], [1, N]]))

                def one(mega, p0, row):
                    ma = mega[:]
                    pstep = ma.ap[0][0]
                    eng = nc.sync if _ring[0] % 2 == 0 else nc.scalar
                    _ring[0] += 1
                    if not do_bc:
                        return
                    # stage 1: mcs row -> 16 copies at partition stride 4
                    src1 = bass.AP(msa.tensor, msa.offset + 7 * row * mstep,
                                   [[mstep, 1], [0, 16], [1, N]])
                    dst1 = bass.AP(ma.tensor, ma.offset + p0 * pstep,
                                   [[4 * pstep, 16], [0, 1], [1, N]])
                    eng.dma_start(dst1, src1)
                    # stage 2: fill j=1..3 within each stride-4 block
                    for j in (1, 2, 3):
                        eng = nc.sync if _ring[0] % 2 == 0 else nc.scalar
                        _ring[0] += 1
                        src2 = bass.AP(ma.tensor, ma.offset + p0 * pstep,
                                       [[4 * pstep, 16], [0, 1], [1, N]])
                        dst2 = bass.AP(ma.tensor,
                                       ma.offset + (p0 + j) * pstep,
                                       [[4 * pstep, 16], [0, 1], [1, N]])
                        eng.dma_start(dst2, src2)

                for g, (taps, _, _, _, _) in enumerate(GROUPS[:4]):
                    kA, kB = taps
                    one(vmeg[g], 0, 2 * kA)
                    one(vmeg[g], 64, 2 * kB)
                    one(hmeg[g], 0, 2 * kA + 1)
                    one(hmeg[g], 64, 2 * kB + 1)
                one(vmeg8, 0, 16)
                one(hmeg8, 0, 17)

            def mult(ci, p):
                """Per-lag-t vertical hat pass (weights dest-aligned, column
                shift folded into the X read), one hm multiply, PE sums t."""
                xsrc = {"XB": XB[p][0:128, 0:SROWS * WP],
                        "XC": XC[p][0:128, 0:SROWS * WP],
                        "XBl": XB[p][0:64, 0:SROWS * WP]}
                vms = [vmeg[g] if g < 4 else vmeg8 for g in range(5)]
                hms = [hmeg[g] if g < 4 else hmeg8 for g in range(5)]

                pot = pso.tile([64, NINT], f32, tag="pot")
                nmm = 2 * NLAG * len(GROUPS)
                mmi = 0
                pool_si = set(range(5 - pool_adds, 5)) if pool_adds else set()
                for g, (taps, xk, ky, ub, Pn) in enumerate(GROUPS):
                    for ti, t in enumerate(LAGS):
                        X = xsrc[xk]
                        X3 = X.rearrange("p (r w) -> p r w", w=WP)
                        vm, hm = vms[g], hms[g]
                        u = ub + t
                        Vt = vp.tile([128, NDQ], bf16, tag="V", name="V")
                        V3 = Vt[:].rearrange("p (r w) -> p r w", w=W)
                        tmps = {}
                        if not do_mul:
                            continue
                        for si in sorted(pool_si):
                            r0 = 2 + ky + LAGS[si]
                            tt = tp.tile([128, NDQ], bf16, tag="tmp")
                            t3 = tt[:].rearrange("p (r w) -> p r w", w=W)
                            nc.gpsimd.tensor_tensor(
                                t3[0:Pn],
                                vm[0:Pn, si * NDQ:(si + 1) * NDQ].rearrange(
                                    "p (r w) -> p r w", w=W),
                                X3[0:Pn, r0:r0 + CH, 4 + u:4 + u + W], MUL)
                            tmps[si] = tt
                        first = True
                        for si in range(NLAG):
                            r0 = 2 + ky + LAGS[si]
                            vm3 = vm[0:Pn, si * NDQ:(si + 1) * NDQ].rearrange(
                                "p (r w) -> p r w", w=W)
                            if first and si not in pool_si:
                                nc.vector.tensor_tensor(
                                    V3[0:Pn], vm3,
                                    X3[0:Pn, r0:r0 + CH, 4 + u:4 + u + W], MUL)
                                first = False
                                continue
                            if si in pool_si:
                                tt = tmps[si]
                            else:
                                tt = tp.tile([128, NDQ], bf16, tag="tmp")
                                nc.vector.tensor_tensor(
                                    tt[:].rearrange("p (r w) -> p r w", w=W)[0:Pn],
                                    vm3,
                                    X3[0:Pn, r0:r0 + CH, 4 + u:4 + u + W], MUL)
                            nc.vector.tensor_tensor(Vt[0:Pn, :], Vt[0:Pn, :],
                                                    tt[0:Pn, :], ADD)
                        if g == 4:
                            q = q8
                            lhs = twdef8[:, 0:64] if ti == 0 else twdef8[:, 64:128]
                            Pq = 65
                        else:
                            q = qp.tile([128, NDQ], bf16, tag="q")
                            lhs = twdefP[:, g * 64:(g + 1) * 64]
                            Pq = Pn
                        nc.vector.tensor_tensor(
                            q[0:Pn, :],
                            hm[0:Pn, ti * NDQ:(ti + 1) * NDQ],
                            Vt[0:Pn, :], MUL)
                        qr = q[:].rearrange("p (r w) -> p r w", w=W)
                        for colh in range(2 if do_mm else 0):
                            nc.tensor.matmul(
                                pot[:, colh * 512:(colh + 1) * 512],
                                lhs,
                                qr[0:Pq, colh * (CH // 2):(colh + 1) * (CH // 2), :],
                                start=(mmi < 2), stop=(mmi >= nmm - 2))
                            mmi += 1
                        if not do_mm:
                            mmi += 2

                if do_mm and do_mul:
                    oe = oep.tile([64, NINT], f32, tag="oe")
                    nc.scalar.activation(oe[:], pot[:], AF.Copy,
                                         bias=0.0, scale=1.0)
                    yo = ci * NINT if isinstance(ci, int) else ci * NINT
                    nc.scalar.dma_start(yout.ap()[:, _sl(yo, NINT)], oe[:])

            with tc.For_i(0, rep, 1):
                prep(0, 0)
                with tc.For_i(0, NCH, 2,
                              hint_engines=(mybir.EngineType.DVE,)) as i:
                    for k in range(2):
                        bcast(k % 2)
                        prep(i + k + 1, (k + 1) % 2)
                        mult(i + k, k % 2)

    return nc


def _split_multiwait(nc, maxw=1):
    """This container's walrus rejects >1 sync-wait per instruction; hoist
    extra waits onto preceding NoOps."""
    n_new = 0
    for f in nc.m.functions:
        for bb in f.blocks:
            out = []
            changed = False
            for ins in bb.instructions:
                si = getattr(ins, "sync_info", None)
                if si is not None and si.on_wait and len(si.on_wait) > maxw:
                    waits = list(si.on_wait)
                    hoist, keep = waits[:-maxw], waits[-maxw:]
                    for i in range(0, len(hoist), maxw):
                        nop = mybir.InstNoOp(
                            name=f"I-waitsplit-{n_new}",
                            sync_info=mybir.SyncInfo(on_wait=hoist[i:i + maxw],
                                                     on_update=[]),
                            bass_nofuse=True,
                            engine=ins.engine)
                        n_new += 1
                        out.append(nop)
                    ins.sync_info = mybir.SyncInfo(on_wait=keep,
                                                  on_update=list(si.on_update))
                    changed = True
                out.append(ins)
            if changed:
                bb.instructions = out
    return n_new


_PROGRAM_CACHE = {}


def _get_program(rep=1):
    if rep not in _PROGRAM_CACHE:
        nc = build_program(rep=rep)
        _split_multiwait(nc)
        _PROGRAM_CACHE[rep] = nc
    return _PROGRAM_CACHE[rep]


def _host_pack(x, w_off, b_off, w_def, b_def):
    slabs = np.zeros((8, 64, XROWS, WP), BF)
    for i in range(8):
        b, r0 = i // 2, (i % 2) * RH
        lo = r0 - 3
        s_lo, s_hi = max(lo, 0), min(lo + 72, H)
        slabs[i, :, s_lo - lo:s_hi - lo, 4:4 + W] = x[b, :, s_lo:s_hi, :].astype(BF)

    wof = w_off.reshape(18, 64, 3, 3)
    woffA = np.zeros((128, 3, 18), BF)
    woffB = np.zeros((64, 3, 18), BF)
    for ctx in range(3):
        woffA[:64, ctx, :] = wof[:, :, 0, ctx].T.astype(BF)
        woffA[64:, ctx, :] = wof[:, :, 1, ctx].T.astype(BF)
        woffB[:, ctx, :] = wof[:, :, 2, ctx].T.astype(BF)

    hb = np.zeros((18, NLAG), np.float32)
    for si, s in enumerate(LAGS):
        hb[:, si] = b_off - s

    wd = w_def.reshape(CO, C, KK)
    wdefP = np.zeros((128, 4, 64), BF)
    for g, (taps, _, _, _, _) in enumerate(GROUPS[:4]):
        kA, kB = taps
        wdefP[:64, g, :] = wd[:, :, kA].T.astype(BF)
        wdefP[64:, g, :] = wd[:, :, kB].T.astype(BF)
    wdef8 = np.zeros((65, 2, 64), BF)
    wdef8[:64, 0, :] = wd[:, :, 8].T.astype(BF)
    wdef8[64, 0, :] = b_def.astype(BF)
    wdef8[:64, 1, :] = wd[:, :, 8].T.astype(BF)

    return [{
        "xs": np.ascontiguousarray(slabs[i].reshape(64, XROWS * WP)),
        "woffA": np.ascontiguousarray(woffA.reshape(128, 54)),
        "woffB": np.ascontiguousarray(woffB.reshape(64, 54)),
        "hbias": hb,
        "wdefP": np.ascontiguousarray(wdefP.reshape(128, 256)),
        "wdef8": np.ascontiguousarray(wdef8.reshape(65, 128)),
    } for i in range(8)]


def kernel(x, w_off, b_off, w_def, b_def):
    x = np.asarray(x, np.float32)
    w_off = np.asarray(w_off, np.float32)
    b_off = np.asarray(b_off, np.float32)
    w_def = np.asarray(w_def, np.float32)
    b_def = np.asarray(b_def, np.float32)

    nc = _get_program(rep=1)
    in_maps = _host_pack(x, w_off, b_off, w_def, b_def)
    res = bass_utils.run_bass_kernel_spmd(nc, in_maps, core_ids=list(range(8)))

    y = np.zeros((B, CO, H, W), np.float32)
    for i in range(8):
        b, r0 = i // 2, (i % 2) * RH
        y[b, :, r0:r0 + RH, :] = res.results[i]["y"].reshape(CO, RH, W)
    return y


if __name__ == "__main__":
    import reference as R
    inp = {k: np.asarray(v, np.float32) for k, v in R.setup_inputs().items()}
    y = kernel(**inp)
    print("out", y.shape, y.dtype, float(np.abs(y).max()))
